# revision 32
# baseline (speedup 1.0000x reference)
"""Single-head causal self-attention (B=4, T=4096, C=1024, H=64) on 8 trn2 cores.

v4: bf16 datapath, host-side x transpose, 256-row q sub-blocks.

Sharding: core = (b, h) with b = core >> 1, h = core & 1. Batch b is data-parallel;
within a batch the two cores split every 512-row block: core h owns rows
[512j+256h, 512j+256h+256) for j = 0..7. The host builds x^T [C, T] in bf16 with
columns ordered group-by-group, OWN 256-half first within each 512 group. With
that ordering the causal band structure is core-independent except for one 0/1
scalar: for sub-block j, s-chunks 4j+0/4j+1 (own half) are triangular, and
chunks 4j+2/4j+3 (the pair core's half) are all-or-nothing (amb = h).

Padded causal area = sum_j (4j+4) = 144 s-chunks of [128, 256] per core
(12.5% over the ideal causal half), vs 160 for the 512-block scheme.

Per-core dataflow (all matmuls bf16, PSUM fp32):
  x^T stripes DMA'd per 512-col group -> [Wk|Wv]-packed kv projection
  (k^T [64,T]; v via small PE transposes into v_aug [s,65] with a ones column
  for the softmax denominator) and [Wq|Wq] q projection of the own half.
  Attention per own 256-row sub-block j: quads of S^T chunk matmuls
  (contraction h=64, N=256), exp on ScalarE per quad (scale=C^-0.5 folded),
  triangular masks on DVE, amb scalar-mul, PV: acc[65,t] += v_aug^T @ P^T
  accumulating the denominator in row 64; normalize, PE-transpose back, DMA out.

Projection work for group j+1 is emitted interleaved into sub-block j's quad
loop so the PE stays busy while ScalarE chews exp. A burst of dummy matmuls at
the start pre-warms the PE HAM clock gate during the first DMA wait.
"""

import sys

if "/opt/trn_rl_repo" not in sys.path:
    sys.path.insert(0, "/opt/trn_rl_repo")

import numpy as np

import concourse.bass as bass
import concourse.mybir as mybir
from concourse import bacc
from concourse.tile import TileContext
from concourse.masks import make_identity

B, T, C, H = 4, 4096, 1024, 64
NCORES = 8
GB = 512            # projection group size (columns of x^T per stripe)
QB = 256            # q sub-block rows
NSB = 8             # sub-blocks per core
SC = 128            # s-chunk size
NCC = C // 128      # 8 contraction chunks
F32 = mybir.dt.float32
BF16 = mybir.dt.bfloat16
SCALE = float(C) ** -0.5

_CACHED_NC = {}


def build_module():
    nc = bacc.Bacc("TRN2", target_bir_lowering=False)
    xt_d = nc.dram_tensor("xt", [C, T], BF16, kind="ExternalInput")
    wkv_d = nc.dram_tensor("wkv", [C, 128], BF16, kind="ExternalInput")
    wqq_d = nc.dram_tensor("wqq", [C, 128], BF16, kind="ExternalInput")
    amb_d = nc.dram_tensor("amb", [128, 1], F32, kind="ExternalInput")
    out_d = nc.dram_tensor("out", [T // 2, H], F32, kind="ExternalOutput")

    with TileContext(nc) as tc:
        with (
            tc.tile_pool(name="const", bufs=1) as const,
            tc.tile_pool(name="xin", bufs=3) as xin,
            tc.tile_pool(name="proj", bufs=1) as proj,
            tc.tile_pool(name="vstage", bufs=2) as vstage,
            tc.tile_pool(name="ptp", bufs=3) as ptp,
            tc.tile_pool(name="outp", bufs=2) as outp,
            tc.tile_pool(name="ps_tr", bufs=1, space="PSUM") as ps_tr,
            tc.tile_pool(name="ps_kvq", bufs=2, space="PSUM") as ps_kvq,
            tc.tile_pool(name="ps_s", bufs=2, space="PSUM") as ps_s,
            tc.tile_pool(name="ps_acc", bufs=1, space="PSUM") as ps_acc,
        ):
            # ---------------- constants ----------------
            identstage = const.tile([128, 128], F32)
            make_identity(nc, identstage)
            identb = const.tile([128, 128], BF16)
            nc.vector.tensor_copy(out=identb, in_=identstage)
            identf = const.tile([H + 1, H + 1], F32)
            make_identity(nc, identf)

            # tri  [128,128]: keep iff t >= s        (t free, s partition)
            # tri2 [128,256]: keep iff t >= s + 128  (zeros first 128 cols)
            tristage = const.tile([128, 384], F32)
            nc.gpsimd.memset(tristage, 1.0)
            nc.gpsimd.affine_select(
                out=tristage[:, 0:128], in_=tristage[:, 0:128],
                compare_op=mybir.AluOpType.is_ge, fill=0.0,
                base=0, pattern=[[1, 128]], channel_multiplier=-1,
            )
            nc.gpsimd.affine_select(
                out=tristage[:, 128:384], in_=tristage[:, 128:384],
                compare_op=mybir.AluOpType.is_ge, fill=0.0,
                base=-128, pattern=[[1, 256]], channel_multiplier=-1,
            )
            trimask = const.tile([128, 384], BF16)
            nc.vector.tensor_copy(out=trimask, in_=tristage)
            tri = trimask[:, 0:128]
            tri2 = trimask[:, 128:384]

            amb = const.tile([128, 1], F32)
            wkv = const.tile([128, NCC, 128], BF16)
            wqq = const.tile([128, NCC, 128], BF16)

            # ---------------- persistent activations ----------------
            kT = proj.tile([64, T], BF16)
            qT = proj.tile([64, T // 2], BF16)
            vaug = proj.tile([128, T // SC, H + 1], BF16)
            nc.gpsimd.memset(vaug, 1.0)  # ones column at [:, :, H]

            xt_src = xt_d.rearrange("(ci p) t -> p ci t", p=128)  # [128, 8, T]

            def emit_dma(g):
                xt = xin.tile([128, NCC, GB], BF16, tag="x")
                nc.sync.dma_start(
                    out=xt, in_=xt_src[:, :, GB * g:GB * (g + 1)])
                return xt

            def proj_feeders(xt, g):
                """Closures emitting projection work for group g."""
                st = {}

                def kv_mm(lo):
                    def f():
                        if lo == 0:
                            st["kv"] = ps_kvq.tile(
                                [128, GB], F32, tag="kvq", name="kv")
                        kv = st["kv"]
                        for ci in range(lo, lo + 4):
                            nc.tensor.matmul(
                                kv, wkv[:, ci, :], xt[:, ci, :],
                                start=(ci == 0), stop=(ci == NCC - 1),
                            )
                    return f

                def q_mm(lo):
                    def f():
                        if lo == 0:
                            st["q"] = ps_kvq.tile(
                                [64, QB], F32, tag="kvq", name="qp")
                        qp = st["q"]
                        for ci in range(lo, lo + 4):
                            nc.tensor.matmul(
                                qp, wqq[:, ci, 0:64], xt[:, ci, 0:QB],
                                start=(ci == 0), stop=(ci == NCC - 1),
                            )
                    return f

                def kv_post():
                    def f():
                        kv = st["kv"]
                        col = GB * g
                        nc.vector.tensor_copy(out=kT[:, col:col + GB], in_=kv[0:64, :])
                        vt = vstage.tile([64, GB], BF16, tag="v")
                        nc.vector.tensor_copy(out=vt, in_=kv[64:128, :])
                        st["vt"] = vt
                    return f

                def v_tr():
                    def f():
                        vt = st["vt"]
                        vtp = ps_tr.tile([128, 4, H], BF16, tag="tr", name="vtp")
                        for m in range(4):
                            nc.tensor.transpose(
                                vtp[:, m, :], vt[:, 128 * m:128 * (m + 1)],
                                identb[0:64, 0:64],
                            )
                        nc.vector.tensor_copy(
                            out=vaug[:, 4 * g:4 * g + 4, 0:H], in_=vtp,
                        )
                    return f

                def q_post():
                    def f():
                        nc.vector.tensor_copy(
                            out=qT[:, QB * g:QB * (g + 1)], in_=st["q"])
                    return f

                # q items deadline: start of c_sub(g). kv items deadline:
                # band quad of c_sub(g) (their only earliest reader).
                qlist = [q_mm(0), q_mm(4), q_post()]
                kvlist = [kv_mm(0), kv_mm(4), kv_post(), v_tr()]
                return qlist, kvlist

            # Global projection-feeder queue of (group, kind, closure).
            # One feeder pops per quad slot; deadlines are enforced by drains:
            # q(j) before c_sub(j)'s first quad, kv(j) before its band quad.
            fqueue = []

            def pop_feeder():
                if fqueue:
                    _, _, f = fqueue.pop(0)
                    f()

            def drain_feeders(gmax, kinds=("q", "kv")):
                while fqueue and fqueue[0][0] <= gmax and fqueue[0][1] in kinds:
                    _, _, f = fqueue.pop(0)
                    f()

            def emit_c_sub(j):
                """Attention for own sub-block j (q rows QB*j .. QB*(j+1))."""
                nch = 4 * j + 4
                drain_feeders(j - 1)          # anything overdue from earlier
                drain_feeders(j, kinds=("q",))  # qT block j must exist now
                acc = ps_acc.tile([H + 1, QB], F32, tag="acc", name="acc")
                nquads = nch // 4
                qs0 = QB * j
                for qd in range(nquads):
                    band = qd == nquads - 1  # chunks 4j..4j+3
                    if band:
                        drain_feeders(j)
                    stile = ps_s.tile([128, 4, QB], F32, tag="s", name="stile")
                    for m in range(4):
                        ch = 4 * qd + m
                        lo = 128 if (band and m == 1) else 0
                        nc.tensor.matmul(
                            stile[:, m, lo:QB],
                            kT[:, SC * ch:SC * (ch + 1)],
                            qT[:, qs0 + lo:qs0 + QB],
                            start=True, stop=True,
                        )
                    pt = ptp.tile([128, 4, QB], BF16, tag="pt", name="pt")
                    nc.scalar.activation(
                        out=pt, in_=stile,
                        func=mybir.ActivationFunctionType.Exp, scale=SCALE,
                    )
                    if band:
                        nc.vector.tensor_mul(pt[:, 0, 0:128], pt[:, 0, 0:128], tri)
                        nc.vector.tensor_mul(pt[:, 1, :], pt[:, 1, :], tri2)
                        nc.vector.tensor_scalar_mul(pt[:, 2, :], pt[:, 2, :], amb[:, 0:1])
                        nc.vector.tensor_scalar_mul(pt[:, 3, :], pt[:, 3, :], amb[:, 0:1])
                    for m in range(4):
                        ch = 4 * qd + m
                        lo = 128 if (band and m == 1) else 0
                        nc.tensor.matmul(
                            acc[:, lo:QB], vaug[:, ch, :], pt[:, m, lo:QB],
                            start=(qd == 0 and m == 0), stop=(band and m == 3),
                        )
                    if not band:
                        pop_feeder()

                # normalize + transpose back + store
                accs = outp.tile([H + 1, QB], F32, tag="accs")
                nc.vector.tensor_copy(out=accs, in_=acc)
                otp = ps_tr.tile([128, 2, H + 1], F32, tag="tr", name="otp")
                for m in range(2):
                    nc.tensor.transpose(
                        otp[:, m, :], accs[:, 128 * m:128 * (m + 1)], identf,
                    )
                ob = outp.tile([128, 2, H + 1], F32, tag="ob")
                nc.vector.tensor_copy(out=ob, in_=otp)
                of = outp.tile([128, 2, H], F32, tag="of")
                rec = outp.tile([128, 2], F32, tag="rec")
                for m in range(2):
                    nc.vector.reciprocal(rec[:, m:m + 1], ob[:, m, H:H + 1])
                    nc.vector.tensor_scalar_mul(of[:, m, :], ob[:, m, 0:H], rec[:, m:m + 1])
                nc.sync.dma_start(
                    out=out_d[QB * j:QB * (j + 1), :].rearrange("(m p) h -> p m h", p=128),
                    in_=of,
                )

            # ---------------- main schedule ----------------
            # group-0 stripe first (split in two so kv matmuls start at half-DMA)
            xt0 = xin.tile([128, NCC, GB], BF16, tag="x", name="xt0")
            nc.sync.dma_start(out=xt0[:, 0:4, :], in_=xt_src[:, 0:4, 0:GB])
            nc.sync.dma_start(
                out=wkv, in_=wkv_d.rearrange("(ci p) w -> p ci w", p=128))
            nc.sync.dma_start(out=xt0[:, 4:8, :], in_=xt_src[:, 4:8, 0:GB])
            nc.sync.dma_start(
                out=wqq, in_=wqq_d.rearrange("(ci p) w -> p ci w", p=128))
            nc.sync.dma_start(out=amb, in_=amb_d[:, :])

            # HAM pre-warm: dummy PE activity during the initial DMA wait so
            # real matmuls start at 2.4 GHz instead of 1.2.
            warm = ps_s.tile([128, 4, QB], F32, tag="s", name="warm")
            for _ in range(70):
                nc.tensor.matmul(
                    warm[0:64, 0, 0:64], identb[0:64, 0:64], identb[0:64, 0:64],
                    start=True, stop=True,
                )

            ql0, kvl0 = proj_feeders(xt0, 0)
            for f in ql0 + kvl0:
                f()
            for j in range(NSB):
                if j + 1 < NSB:
                    ql, kvl = proj_feeders(emit_dma(j + 1), j + 1)
                    for f in ql:
                        fqueue.append((j + 1, "q", f))
                    for f in kvl:
                        fqueue.append((j + 1, "kv", f))
                emit_c_sub(j)

    nc.compile()
    return nc


def _get_nc():
    if "nc" not in _CACHED_NC:
        _CACHED_NC["nc"] = build_module()
    return _CACHED_NC["nc"]


def make_in_maps(x, wk, wq, wv):
    npbf = mybir.dt.np(BF16)
    wkv = np.ascontiguousarray(np.concatenate([wk, wv], axis=1)).astype(npbf)
    wqq = np.ascontiguousarray(np.concatenate([wq, wq], axis=1)).astype(npbf)
    in_maps = []
    for core in range(NCORES):
        b, h = core >> 1, core & 1
        rows = np.concatenate([
            np.arange(GB * j + QB * hh, GB * j + QB * hh + QB)
            for j in range(NSB) for hh in (h, 1 - h)
        ])
        xt = np.ascontiguousarray(x[b][rows].astype(npbf).T)  # [C, T]
        in_maps.append({
            "xt": xt, "wkv": wkv, "wqq": wqq,
            "amb": np.full((128, 1), float(h), dtype=np.float32),
        })
    return in_maps


def assemble(results):
    out = np.empty((B, T, H), dtype=np.float32)
    for core in range(NCORES):
        b, h = core >> 1, core & 1
        o = results[core]["out"]
        for j in range(NSB):
            r0 = GB * j + QB * h
            out[b, r0:r0 + QB, :] = o[QB * j:QB * (j + 1), :]
    return out


def kernel(x, Wk, Wq, Wv):
    from concourse import bass_utils

    x = np.asarray(x, dtype=np.float32)
    wk = np.ascontiguousarray(np.asarray(Wk, dtype=np.float32))
    wq = np.ascontiguousarray(np.asarray(Wq, dtype=np.float32))
    wv = np.ascontiguousarray(np.asarray(Wv, dtype=np.float32))
    nc = _get_nc()
    in_maps = make_in_maps(x, wk, wq, wv)
    res = bass_utils.run_bass_kernel_spmd(nc, in_maps, core_ids=list(range(NCORES)))
    return assemble(res.results)
